# revision 16
# baseline (speedup 1.0000x reference)
"""Multi-head attention (B=2, S=2048, D=1024, H=16, dk=64) on 8 trn2 cores, v4.

Sharding: core c handles batch b=c//4 and 4 heads g=c%4 (heads 4g..4g+3).

v4 = v3's transposed-score/ones-column pipeline, restructured so ScalarE
(exp) starts ~10us in instead of ~81us:
  - chunked DMA (512-col slices) across sync/gpsimd queues; only
    wk+xk[0:512]+wq+xq[0:512] gate the first score matmuls.
  - prologue computes just khT/qhT pair-0 chunk-0; ALL other projection
    work (K/Q rest, V, pair 1) is deferred into the attention stream's
    per-slot aux budget, scheduled FIFO with deadlines ahead of use.
  - unit order p-outer: all 4 q-groups of pair 0, then pair 1, so
    pair-1 projections can hide under pair-0's attention.
  - exp activation table preloaded via a dummy exp at t=0.
  - exp tiles in an 18-deep ring of [128,1024] tiles (PV of unit u reads
    them one unit behind the exp writes of u+1).
PSUM: sc 2x[128,1024] (4 banks) + ctx 2x[65,512] (2) + aux 2x[128,512]
(2, shared by proj chunks / norm-broadcast / out-proj) = 8 banks.
"""

import sys

for _p in ("/opt/trn_rl_repo",):
    if _p not in sys.path:
        sys.path.insert(0, _p)

from collections import deque
from contextlib import ExitStack

import ml_dtypes
import numpy as np

import concourse.bass as bass
import concourse.bacc as bacc_mod
import concourse.mybir as mybir
import concourse.tile as tile
from concourse.bass_utils import run_bass_kernel_spmd

BF16 = mybir.dt.bfloat16
F32 = mybir.dt.float32
AF = mybir.ActivationFunctionType

B, S, D = 2, 2048, 1024
NCORES = 8
HLOC = 4          # heads per core
DK = 64
HD = HLOC * DK    # local head dims = 256
KT_D = D // 128   # 8 k-tiles over the model dim
NST = S // 128    # 16 tiles over sequence (k-tiles of attention)
NQG = 4           # q groups of 512
CW = 512          # chunk width (S columns) for x DMA + projections
NCH = S // CW     # 4 chunks
VCW = 256         # v chunk width (2 st)
VCH = S // VCW    # 8 v chunks
ERING = 18        # exp tile ring depth (>= 17)


def build_nc() -> bass.Bass:
    nc = bacc_mod.Bacc()

    # x tensors arrive chunk-contiguous (host lays them out so each DMA
    # reads [128 part x 8KB] fully-contiguous blocks at full HBM rate):
    # xqC[p, c, t, n] = x^T[t*128+p, c*CW+n]
    xqC = nc.dram_tensor("xqC", [128, NCH, KT_D, CW], BF16, kind="ExternalInput")
    xkC = nc.dram_tensor("xkC", [128, NCH, KT_D, CW], BF16, kind="ExternalInput")
    xvC = nc.dram_tensor("xvC", [128, VCH, KT_D, VCW], BF16, kind="ExternalInput")
    wqT = nc.dram_tensor("wqT", [D, HD], BF16, kind="ExternalInput")
    wkT = nc.dram_tensor("wkT", [D, HD], BF16, kind="ExternalInput")
    wvT = nc.dram_tensor("wvT", [D, HD], BF16, kind="ExternalInput")
    woT = nc.dram_tensor("woT", [HD, D], BF16, kind="ExternalInput")
    bqd = nc.dram_tensor("bq", [HD], BF16, kind="ExternalInput")
    out = nc.dram_tensor("out_partial", [S, D], F32, kind="ExternalOutput")

    with tile.TileContext(nc) as tc, ExitStack() as ctx:
        const = ctx.enter_context(tc.tile_pool(name="const", bufs=1))
        persist = ctx.enter_context(tc.tile_pool(name="persist", bufs=1))

        wq_s = const.tile([128, KT_D, HD], BF16, tag="wq")
        wk_s = const.tile([128, KT_D, HD], BF16, tag="wk")
        wv_s = const.tile([128, KT_D, HD], BF16, tag="wv")
        wo_s = const.tile([128, 2, D], BF16, tag="wo")
        bq_s = const.tile([1, HD], BF16, tag="bq")
        ones_row = const.tile([1, CW], BF16, tag="ones")
        ones64 = const.tile([128, DK], BF16, tag="ones64")
        scr = const.tile([1, DK], BF16, tag="scr")
        nc.vector.memset(ones_row, 1.0)
        nc.vector.memset(ones64, 1.0)

        qhT = [persist.tile([128, S], BF16, tag=f"qhT{m}", name=f"qhT{m}") for m in range(2)]
        khT = [persist.tile([128, S], BF16, tag=f"khT{m}", name=f"khT{m}") for m in range(2)]
        vh2 = persist.tile([128, NST, HLOC, DK + 1], BF16, tag="vh2")
        ctxT = [persist.tile([128, S], BF16, tag=f"ctxT{m}", name=f"ctxT{m}") for m in range(2)]
        nc.vector.memset(vh2[:, :, :, DK : DK + 1], 1.0)

        # chunk-major x tiles: [:, c, kt, :] is one DMA'd chunk
        xq_t = persist.tile([128, NCH, KT_D, CW], BF16, tag="xq")
        xk_t = persist.tile([128, NCH, KT_D, CW], BF16, tag="xk")

        # Preload the exp table set while DMA streams in.
        nc.scalar.activation(scr, ones_row[0:1, 0:DK], AF.Exp)

        # ---- DMA issues -------------------------------------------------
        # ALL input loads ride ONE queue (gpsimd) in strict priority
        # order: two queues merely split HBM bandwidth and delay the
        # critical path (one queue alone saturates ~400GB/s). The sync
        # queue is reserved for the out-row stores.
        nc.gpsimd.dma_start(wk_s, wkT.rearrange("(t p) n -> p t n", p=128))
        nc.gpsimd.dma_start(xk_t[:, 0], xkC[:, 0])
        nc.gpsimd.dma_start(wq_s, wqT.rearrange("(t p) n -> p t n", p=128))
        nc.gpsimd.dma_start(bq_s, bqd.rearrange("(o n) -> o n", o=1))
        nc.gpsimd.dma_start(xq_t[:, 0], xqC[:, 0])
        nc.gpsimd.dma_start(xk_t[:, 1], xkC[:, 1])
        nc.gpsimd.dma_start(wv_s, wvT.rearrange("(t p) n -> p t n", p=128))

        with (
            tc.tile_pool(name="xvp", bufs=4) as xvp,
            tc.tile_pool(name="sc_psum", bufs=2, space="PSUM") as scp,
            tc.tile_pool(name="ctx_psum", bufs=1, space="PSUM") as ctxp,
            tc.tile_pool(name="aux_psum", bufs=2, space="PSUM") as auxp,
            tc.tile_pool(name="exp_pool", bufs=ERING) as epool,
            tc.tile_pool(name="csb_pool", bufs=2) as csbp,
            tc.tile_pool(name="stat_pool", bufs=3) as stp,
            tc.tile_pool(name="cn_pool", bufs=2) as cnp,
            tc.tile_pool(name="out_sbuf", bufs=2) as obp,
        ):
            # Remaining loads, still one queue, arrival matched to use
            # order. xv slot-reuse waits (chunk c vs V-proj of chunk c-4)
            # only ever block this queue; wo rides last (needed ~100us in).
            xv_tiles = [
                xvp.tile([128, KT_D, VCW], BF16, tag="xv", name=f"xv{c}")
                for c in range(VCH)
            ]
            nc.gpsimd.dma_start(xv_tiles[0], xvC[:, 0])
            nc.gpsimd.dma_start(xq_t[:, 1], xqC[:, 1])
            nc.gpsimd.dma_start(xk_t[:, 2], xkC[:, 2])
            nc.gpsimd.dma_start(xv_tiles[1], xvC[:, 1])
            nc.gpsimd.dma_start(xv_tiles[2], xvC[:, 2])
            nc.gpsimd.dma_start(xq_t[:, 2], xqC[:, 2])
            nc.gpsimd.dma_start(xk_t[:, 3], xkC[:, 3])
            nc.gpsimd.dma_start(xv_tiles[3], xvC[:, 3])
            nc.gpsimd.dma_start(xq_t[:, 3], xqC[:, 3])
            for c in range(4, VCH):
                nc.gpsimd.dma_start(xv_tiles[c], xvC[:, c])
            nc.gpsimd.dma_start(wo_s, woT.rearrange("(t p) n -> p t n", p=128))

            # ---- deferred-work machinery --------------------------------
            pe_aux = deque()  # (mm_cost, closure)
            credits = [0]

            def drain_aux(grant: int, cap: int = 12):
                credits[0] = min(credits[0] + grant, cap)
                while pe_aux and credits[0] >= pe_aux[0][0]:
                    cost, fn = pe_aux.popleft()
                    credits[0] -= cost
                    fn()

            def qk_chunk(dst, wsrc, p: int, xsrc, c: int, bias: bool):
                def run():
                    ps = auxp.tile([128, CW], F32, tag="aux", name="proj")
                    for kt in range(KT_D):
                        nc.tensor.matmul(
                            ps,
                            lhsT=wsrc[:, kt, p * 128 : (p + 1) * 128],
                            rhs=xsrc[:, c, kt],
                            start=(kt == 0),
                            stop=(False if bias else kt == KT_D - 1),
                        )
                    if bias:
                        nc.tensor.matmul(
                            ps,
                            lhsT=bq_s[0:1, p * 128 : (p + 1) * 128],
                            rhs=ones_row[0:1, :],
                            start=False,
                            stop=True,
                        )
                    nc.vector.tensor_copy(dst[:, c * CW : (c + 1) * CW], ps)
                return run

            def v_chunk(c: int):
                def run():
                    ps = auxp.tile([128, 2, HLOC, DK], F32, tag="aux", name="vproj")
                    xvt = xv_tiles[c]
                    for sl in range(2):
                        for kt in range(KT_D):
                            nc.tensor.matmul(
                                ps[:, sl],
                                lhsT=xvt[:, kt, sl * 128 : (sl + 1) * 128],
                                rhs=wv_s[:, kt, :],
                                start=(kt == 0),
                                stop=(kt == KT_D - 1),
                            )
                    nc.vector.tensor_copy(vh2[:, 2 * c : 2 * c + 2, :, 0:DK], ps)
                return run

            def outproj_closure(st: int):
                def run():
                    ob = obp.tile([128, D], F32, tag="ob", name="ob")
                    for nb in range(2):
                        ops = auxp.tile([128, 512], F32, tag="aux", name="ops")
                        for ktp in range(2):
                            nc.tensor.matmul(
                                ops,
                                lhsT=ctxT[ktp][:, st * 128 : (st + 1) * 128],
                                rhs=wo_s[:, ktp, nb * 512 : (nb + 1) * 512],
                                start=(ktp == 0),
                                stop=(ktp == 1),
                            )
                        nc.vector.tensor_copy(ob[:, nb * 512 : (nb + 1) * 512], ops)
                    nc.sync.dma_start(out[st * 128 : (st + 1) * 128, :], ob)
                return run

            def norm_closure(qg: int, p: int, hh: int, cps_sb, statb):
                q0 = qg * 512

                def run():
                    rep = auxp.tile([128, 512], F32, tag="aux", name="rep")
                    nc.tensor.matmul(
                        rep[0:DK, :],
                        lhsT=ones64[DK : DK + 1, :],
                        rhs=statb[DK : DK + 1, :],
                        start=True,
                        stop=True,
                        tile_position=(64, 0),
                    )
                    if hh == 0:
                        nc.vector.tensor_mul(
                            ctxT[p][0:DK, q0 : q0 + 512], cps_sb[0:DK, :], rep[0:DK, :]
                        )
                    else:
                        cn = cnp.tile([DK, 512], BF16, tag="cn", name="cn")
                        nc.vector.tensor_mul(cn, cps_sb[0:DK, :], rep[0:DK, :])
                        # gpsimd queue: idle after the xv/wo loads, keeps the
                        # sync queue free for the out-row DMAs.
                        nc.gpsimd.dma_start(ctxT[p][DK:128, q0 : q0 + 512], cn)
                        if p == 1:
                            for st in range(qg * 4, qg * 4 + 4):
                                pe_aux.append((4, outproj_closure(st)))
                return run

            def pv_slot(prev, kt: int):
                _, pp, pexps, pcps = prev
                for hh in range(2):
                    nc.tensor.matmul(
                        pcps[hh],
                        lhsT=vh2[:, kt, pp * 2 + hh, :],
                        rhs=pexps[kt][:, hh * 512 : (hh + 1) * 512],
                        start=(kt == 0),
                        stop=(kt == NST - 1),
                    )

            def finish_norm(prev):
                pqg, pp, _, pcps = prev
                for hh in range(2):
                    cps_sb = csbp.tile([DK + 1, 512], F32, tag=f"csb{hh}", name="cps_sb")
                    nc.vector.tensor_copy(cps_sb, pcps[hh])
                    stat = stp.tile([DK + 1, 512], F32, tag="stat", name="stat")
                    # NB: reciprocal_approx_fast mis-addresses single-row APs at
                    # base partition 64; run it over the full tile (same cost --
                    # partitions are parallel) and use only row 64.
                    nc.vector.reciprocal_approx_fast(stat, cps_sb)
                    statb = stp.tile([DK + 1, 512], BF16, tag="statb", name="statb")
                    nc.vector.tensor_copy(statb[DK : DK + 1, :], stat[DK : DK + 1, :])
                    pe_aux.append((1, norm_closure(pqg, pp, hh, cps_sb, statb)))

            # ---- prologue -----------------------------------------------
            # Warm the PE HAM (4096-cycle activity window) with throwaway
            # matmuls while the first x chunks stream in, so the real
            # projections run at 2.4GHz instead of 1.2.
            warm = auxp.tile([128, 512], F32, tag="aux", name="warm")
            for _ in range(56):
                nc.tensor.matmul(
                    warm[0:DK, 0:DK], lhsT=ones64, rhs=ones64, start=True, stop=True
                )
            # pair-0 chunk-0 projections gate the first scores
            qk_chunk(khT[0], wk_s, 0, xk_t, 0, bias=False)()
            qk_chunk(qhT[0], wq_s, 0, xq_t, 0, bias=True)()

            # ---- deferred projection schedule (FIFO, deadline-ordered) --
            sched = [
                (8, qk_chunk(khT[0], wk_s, 0, xk_t, 1, False)),
                (8, qk_chunk(khT[0], wk_s, 0, xk_t, 2, False)),
                (8, qk_chunk(khT[0], wk_s, 0, xk_t, 3, False)),
                (9, qk_chunk(qhT[0], wq_s, 0, xq_t, 1, True)),
                (8, v_chunk(0)),
                (8, v_chunk(1)),
                (9, qk_chunk(qhT[0], wq_s, 0, xq_t, 2, True)),
                (8, v_chunk(2)),
                (8, v_chunk(3)),
                (8, v_chunk(4)),
                (8, v_chunk(5)),
                (8, v_chunk(6)),
                (8, v_chunk(7)),
                (9, qk_chunk(qhT[0], wq_s, 0, xq_t, 3, True)),
                (8, qk_chunk(khT[1], wk_s, 1, xk_t, 0, False)),
                (8, qk_chunk(khT[1], wk_s, 1, xk_t, 1, False)),
                (8, qk_chunk(khT[1], wk_s, 1, xk_t, 2, False)),
                (8, qk_chunk(khT[1], wk_s, 1, xk_t, 3, False)),
                (9, qk_chunk(qhT[1], wq_s, 1, xq_t, 0, True)),
                (9, qk_chunk(qhT[1], wq_s, 1, xq_t, 1, True)),
                (9, qk_chunk(qhT[1], wq_s, 1, xq_t, 2, True)),
                (9, qk_chunk(qhT[1], wq_s, 1, xq_t, 3, True)),
            ]
            pe_aux.extend(sched)

            # ---- attention units, software-pipelined --------------------
            units = [(qg, p) for p in range(2) for qg in range(NQG)]
            # In late (p=1) units, drain the previous unit's PV at 1.5
            # kts/slot so its norm + out-projection run in-unit instead of
            # piling into the kernel tail.
            fast_pv = {
                0: [0, 1], 1: [2], 2: [3, 4], 3: [5], 4: [6, 7], 5: [8],
                6: [9, 10], 7: [11], 8: [12, 13], 9: [14], 10: [15],
            }
            prev = None
            for ui, (qg, p) in enumerate(units):
                q0 = qg * 512
                grant = 4 if (ui <= 1 or ui >= 6) else 3
                fast = ui >= 5
                prev_done = False
                exps = []
                for kt in range(NST):
                    sc = scp.tile([128, 1024], F32, tag="sc", name="sc")
                    nc.tensor.matmul(
                        sc[:, 0:512],
                        lhsT=khT[p][0:64, kt * 128 : (kt + 1) * 128],
                        rhs=qhT[p][0:64, q0 : q0 + 512],
                        start=True,
                        stop=True,
                        tile_position=(0, 0),
                    )
                    nc.tensor.matmul(
                        sc[:, 512:1024],
                        lhsT=khT[p][64:128, kt * 128 : (kt + 1) * 128],
                        rhs=qhT[p][64:128, q0 : q0 + 512],
                        start=True,
                        stop=True,
                        tile_position=(64, 0),
                    )
                    et = epool.tile([128, 1024], BF16, tag="expT", name="expT")
                    nc.scalar.activation(et, sc, AF.Exp)
                    exps.append(et)
                    if prev is not None:
                        pkts = fast_pv.get(kt, []) if fast else [kt]
                        for pkt in pkts:
                            pv_slot(prev, pkt)
                        if fast and kt == 10:
                            finish_norm(prev)
                            prev_done = True
                    drain_aux(grant)
                if prev is not None and not prev_done:
                    finish_norm(prev)
                if ui == len(units) - 1:
                    # Last unit: take ctx PSUM from the (now idle) score
                    # pool so the tail's PV never waits on the previous
                    # unit's ctx-bank release chain.
                    cps = [
                        scp.tile([DK + 1, 512], F32, tag="sc", name="cps_tail")
                        for hh in range(2)
                    ]
                else:
                    cps = [
                        ctxp.tile([DK + 1, 512], F32, tag=f"c{hh}", name="cps")
                        for hh in range(2)
                    ]
                prev = (qg, p, exps, cps)

            # Tail: PV + norm + remaining out projections of the last unit.
            for kt in range(NST):
                pv_slot(prev, kt)
                drain_aux(3)
            finish_norm(prev)
            while pe_aux:
                pe_aux.popleft()[1]()

    nc.compile()
    return nc


_CACHE: dict = {}


def _get_nc() -> bass.Bass:
    if "nc" not in _CACHE:
        _CACHE["nc"] = build_nc()
    return _CACHE["nc"]


def _bf16(x: np.ndarray) -> np.ndarray:
    return np.ascontiguousarray(x).astype(ml_dtypes.bfloat16)


def _chunked(xT: np.ndarray, nch: int, cw: int) -> np.ndarray:
    # x^T [D, S] -> [128, nch, KT_D, cw] with [p, c, t, n] = x^T[t*128+p, c*cw+n]
    return np.ascontiguousarray(
        xT.reshape(KT_D, 128, nch, cw).transpose(1, 2, 0, 3)
    )


def make_in_maps(q, k, v, wq, bq, wk, bk, wv, bv, wo, bo):
    scale = np.float32(1.0 / np.sqrt(DK))
    in_maps = []
    xT = {}
    for b in range(B):
        xT[b] = (
            _chunked(_bf16(np.asarray(q[b], np.float32).T), NCH, CW),
            _chunked(_bf16(np.asarray(k[b], np.float32).T), NCH, CW),
            _chunked(_bf16(np.asarray(v[b], np.float32).T), VCH, VCW),
        )
    for c in range(NCORES):
        b, g = divmod(c, 4)
        hh = g * HD
        in_maps.append(
            {
                "xqC": xT[b][0],
                "xkC": xT[b][1],
                "xvC": xT[b][2],
                "wqT": _bf16(np.asarray(wq[hh : hh + HD], np.float32).T * scale),
                "wkT": _bf16(np.asarray(wk[hh : hh + HD], np.float32).T),
                "wvT": _bf16(np.asarray(wv[hh : hh + HD], np.float32).T),
                "woT": _bf16(np.asarray(wo[:, hh : hh + HD], np.float32).T),
                "bq": _bf16(np.asarray(bq[hh : hh + HD], np.float32) * scale),
            }
        )
    return in_maps


def assemble(results, bv, bo, wo) -> np.ndarray:
    out = np.zeros((B, S, D), np.float32)
    for c in range(NCORES):
        out[c // 4] += np.asarray(results[c]["out_partial"], np.float32)
    corr = np.asarray(bv, np.float32) @ np.asarray(wo, np.float32).T + np.asarray(
        bo, np.float32
    )
    out += corr[None, None, :]
    return out


def kernel(q, k, v, wq, bq, wk, bk, wv, bv, wo, bo) -> np.ndarray:
    nc = _get_nc()
    in_maps = make_in_maps(q, k, v, wq, bq, wk, bk, wv, bv, wo, bo)
    res = run_bass_kernel_spmd(nc, in_maps, list(range(NCORES))).results
    return assemble(res, bv, bo, wo)


# revision 24
# speedup vs baseline: 1.0169x; 1.0169x over previous
"""Multi-head attention (B=2, S=2048, D=1024, H=16, dk=64) on 8 trn2 cores, v4.

Sharding: core c handles batch b=c//4 and 4 heads g=c%4 (heads 4g..4g+3).

v4 = v3's transposed-score/ones-column pipeline, restructured so ScalarE
(exp) starts ~10us in instead of ~81us:
  - chunked DMA (512-col slices) across sync/gpsimd queues; only
    wk+xk[0:512]+wq+xq[0:512] gate the first score matmuls.
  - prologue computes just khT/qhT pair-0 chunk-0; ALL other projection
    work (K/Q rest, V, pair 1) is deferred into the attention stream's
    per-slot aux budget, scheduled FIFO with deadlines ahead of use.
  - unit order p-outer: all 4 q-groups of pair 0, then pair 1, so
    pair-1 projections can hide under pair-0's attention.
  - exp activation table preloaded via a dummy exp at t=0.
  - exp tiles in an 18-deep ring of [128,1024] tiles (PV of unit u reads
    them one unit behind the exp writes of u+1).
PSUM: sc 2x[128,1024] (4 banks) + ctx 2x[65,512] (2) + aux 2x[128,512]
(2, shared by proj chunks / norm-broadcast / out-proj) = 8 banks.
"""

import sys

for _p in ("/opt/trn_rl_repo",):
    if _p not in sys.path:
        sys.path.insert(0, _p)

from collections import deque
from contextlib import ExitStack

import ml_dtypes
import numpy as np

import concourse.bass as bass
import concourse.bacc as bacc_mod
import concourse.mybir as mybir
import concourse.tile as tile
from concourse.bass_utils import run_bass_kernel_spmd

BF16 = mybir.dt.bfloat16
F32 = mybir.dt.float32
AF = mybir.ActivationFunctionType

B, S, D = 2, 2048, 1024
NCORES = 8
HLOC = 4          # heads per core
DK = 64
HD = HLOC * DK    # local head dims = 256
KT_D = D // 128   # 8 k-tiles over the model dim
NST = S // 128    # 16 tiles over sequence (k-tiles of attention)
NQG = 4           # q groups of 512
CW = 512          # chunk width (S columns) for x DMA + projections
NCH = S // CW     # 4 chunks
VCW = 256         # v chunk width (2 st)
VCH = S // VCW    # 8 v chunks
ERING = 18        # exp tile ring depth (>= 17)


def build_nc() -> bass.Bass:
    nc = bacc_mod.Bacc()

    # x tensors arrive chunk-contiguous (host lays them out so each DMA
    # reads [128 part x 8KB] fully-contiguous blocks at full HBM rate):
    # xqC[p, c, t, n] = x^T[t*128+p, c*CW+n]
    xqC = nc.dram_tensor("xqC", [128, NCH, KT_D, CW], BF16, kind="ExternalInput")
    xkC = nc.dram_tensor("xkC", [128, NCH, KT_D, CW], BF16, kind="ExternalInput")
    xvC = nc.dram_tensor("xvC", [128, VCH, KT_D, VCW], BF16, kind="ExternalInput")
    wqT = nc.dram_tensor("wqT", [D, HD], BF16, kind="ExternalInput")
    wkT = nc.dram_tensor("wkT", [D, HD], BF16, kind="ExternalInput")
    wvT = nc.dram_tensor("wvT", [D, HD], BF16, kind="ExternalInput")
    woT = nc.dram_tensor("woT", [HD, D], BF16, kind="ExternalInput")
    bqd = nc.dram_tensor("bq", [HD], BF16, kind="ExternalInput")
    out = nc.dram_tensor("out_partial", [S, D], F32, kind="ExternalOutput")

    with tile.TileContext(nc) as tc, ExitStack() as ctx:
        const = ctx.enter_context(tc.tile_pool(name="const", bufs=1))
        persist = ctx.enter_context(tc.tile_pool(name="persist", bufs=1))

        wq_s = const.tile([128, KT_D, HD], BF16, tag="wq")
        wk_s = const.tile([128, KT_D, HD], BF16, tag="wk")
        wv_s = const.tile([128, KT_D, HD], BF16, tag="wv")
        wo_s = const.tile([128, 2, D], BF16, tag="wo")
        bq_s = const.tile([1, HD], BF16, tag="bq")
        ones_row = const.tile([1, CW], BF16, tag="ones")
        ones64 = const.tile([128, DK], BF16, tag="ones64")
        scr = const.tile([1, DK], BF16, tag="scr")
        nc.vector.memset(ones_row, 1.0)
        nc.vector.memset(ones64, 1.0)

        qhT = [persist.tile([128, S], BF16, tag=f"qhT{m}", name=f"qhT{m}") for m in range(2)]
        khT = [persist.tile([128, S], BF16, tag=f"khT{m}", name=f"khT{m}") for m in range(2)]
        vh2 = persist.tile([128, NST, HLOC, DK + 1], BF16, tag="vh2")
        ctxT = [persist.tile([128, S], BF16, tag=f"ctxT{m}", name=f"ctxT{m}") for m in range(2)]
        nc.vector.memset(vh2[:, :, :, DK : DK + 1], 1.0)

        # chunk-major x tiles: [:, c, kt, :] is one DMA'd chunk
        xq_t = persist.tile([128, NCH, KT_D, CW], BF16, tag="xq")
        xk_t = persist.tile([128, NCH, KT_D, CW], BF16, tag="xk")

        # Preload the exp table set while DMA streams in.
        nc.scalar.activation(scr, ones_row[0:1, 0:DK], AF.Exp)

        # ---- DMA issues -------------------------------------------------
        # ALL input loads ride ONE queue (gpsimd) in strict priority
        # order: two queues merely split HBM bandwidth and delay the
        # critical path (one queue alone saturates ~400GB/s). The sync
        # queue is reserved for the out-row stores.
        nc.gpsimd.dma_start(wk_s, wkT.rearrange("(t p) n -> p t n", p=128))
        nc.gpsimd.dma_start(xk_t[:, 0], xkC[:, 0])
        nc.gpsimd.dma_start(wq_s, wqT.rearrange("(t p) n -> p t n", p=128))
        nc.gpsimd.dma_start(bq_s, bqd.rearrange("(o n) -> o n", o=1))
        nc.gpsimd.dma_start(xq_t[:, 0], xqC[:, 0])
        nc.gpsimd.dma_start(xk_t[:, 1], xkC[:, 1])
        nc.gpsimd.dma_start(wv_s, wvT.rearrange("(t p) n -> p t n", p=128))

        with (
            tc.tile_pool(name="xvp", bufs=4) as xvp,
            tc.tile_pool(name="sc_psum", bufs=2, space="PSUM") as scp,
            tc.tile_pool(name="ctx_psum", bufs=1, space="PSUM") as ctxp,
            tc.tile_pool(name="aux_psum", bufs=2, space="PSUM") as auxp,
            tc.tile_pool(name="exp_pool", bufs=ERING) as epool,
            tc.tile_pool(name="csb_pool", bufs=2) as csbp,
            tc.tile_pool(name="stat_pool", bufs=3) as stp,
            tc.tile_pool(name="cn_pool", bufs=2) as cnp,
            tc.tile_pool(name="out_sbuf", bufs=2) as obp,
        ):
            # Remaining loads, still one queue, arrival matched to use
            # order. xv slot-reuse waits (chunk c vs V-proj of chunk c-4)
            # only ever block this queue; wo rides last (needed ~100us in).
            xv_tiles = [
                xvp.tile([128, KT_D, VCW], BF16, tag="xv", name=f"xv{c}")
                for c in range(VCH)
            ]
            nc.gpsimd.dma_start(xk_t[:, 2], xkC[:, 2])
            nc.gpsimd.dma_start(xk_t[:, 3], xkC[:, 3])
            nc.gpsimd.dma_start(xv_tiles[0], xvC[:, 0])
            nc.gpsimd.dma_start(xq_t[:, 1], xqC[:, 1])
            nc.gpsimd.dma_start(xv_tiles[1], xvC[:, 1])
            nc.gpsimd.dma_start(xv_tiles[2], xvC[:, 2])
            nc.gpsimd.dma_start(xq_t[:, 2], xqC[:, 2])
            nc.gpsimd.dma_start(xv_tiles[3], xvC[:, 3])
            nc.gpsimd.dma_start(xq_t[:, 3], xqC[:, 3])
            for c in range(4, VCH):
                nc.gpsimd.dma_start(xv_tiles[c], xvC[:, c])
            nc.gpsimd.dma_start(wo_s, woT.rearrange("(t p) n -> p t n", p=128))

            # ---- deferred-work machinery --------------------------------
            pe_aux = deque()  # (mm_cost, min_unit, closure)
            credits = [0]
            cur_ui = [0]

            def drain_aux(grant: int, cap: int = 12):
                credits[0] = min(credits[0] + grant, cap)
                while (
                    pe_aux
                    and credits[0] >= pe_aux[0][0]
                    and pe_aux[0][1] <= cur_ui[0]
                ):
                    cost, _, fn = pe_aux.popleft()
                    credits[0] -= cost
                    fn()

            def qk_chunk(dst, wsrc, p: int, xsrc, c: int, bias: bool):
                def run():
                    ps = auxp.tile([128, CW], F32, tag="aux", name="proj")
                    for kt in range(KT_D):
                        nc.tensor.matmul(
                            ps,
                            lhsT=wsrc[:, kt, p * 128 : (p + 1) * 128],
                            rhs=xsrc[:, c, kt],
                            start=(kt == 0),
                            stop=(False if bias else kt == KT_D - 1),
                        )
                    if bias:
                        nc.tensor.matmul(
                            ps,
                            lhsT=bq_s[0:1, p * 128 : (p + 1) * 128],
                            rhs=ones_row[0:1, :],
                            start=False,
                            stop=True,
                        )
                    nc.vector.tensor_copy(dst[:, c * CW : (c + 1) * CW], ps)
                return run

            def v_chunk(c: int):
                def run():
                    ps = auxp.tile([128, 2, HLOC, DK], F32, tag="aux", name="vproj")
                    xvt = xv_tiles[c]
                    for sl in range(2):
                        for kt in range(KT_D):
                            nc.tensor.matmul(
                                ps[:, sl],
                                lhsT=xvt[:, kt, sl * 128 : (sl + 1) * 128],
                                rhs=wv_s[:, kt, :],
                                start=(kt == 0),
                                stop=(kt == KT_D - 1),
                            )
                    nc.vector.tensor_copy(vh2[:, 2 * c : 2 * c + 2, :, 0:DK], ps)
                return run

            def outproj_closure(st: int):
                def run():
                    ob = obp.tile([128, D], F32, tag="ob", name="ob")
                    for nb in range(2):
                        ops = auxp.tile([128, 512], F32, tag="aux", name="ops")
                        for ktp in range(2):
                            nc.tensor.matmul(
                                ops,
                                lhsT=ctxT[ktp][:, st * 128 : (st + 1) * 128],
                                rhs=wo_s[:, ktp, nb * 512 : (nb + 1) * 512],
                                start=(ktp == 0),
                                stop=(ktp == 1),
                            )
                        nc.vector.tensor_copy(ob[:, nb * 512 : (nb + 1) * 512], ops)
                    nc.sync.dma_start(out[st * 128 : (st + 1) * 128, :], ob)
                return run

            def norm_closure(qg: int, p: int, hh: int, cps_sb, statb):
                q0 = qg * 512

                def run():
                    rep = auxp.tile([128, 512], F32, tag="aux", name="rep")
                    nc.tensor.matmul(
                        rep[0:DK, :],
                        lhsT=ones64[DK : DK + 1, :],
                        rhs=statb[DK : DK + 1, :],
                        start=True,
                        stop=True,
                        tile_position=(64, 0),
                    )
                    if hh == 0:
                        nc.vector.tensor_mul(
                            ctxT[p][0:DK, q0 : q0 + 512], cps_sb[0:DK, :], rep[0:DK, :]
                        )
                    else:
                        cn = cnp.tile([DK, 512], BF16, tag="cn", name="cn")
                        nc.vector.tensor_mul(cn, cps_sb[0:DK, :], rep[0:DK, :])
                        # gpsimd queue (idle after the xv/wo loads) keeps the
                        # sync queue free for out-row DMAs; the LAST unit's
                        # write rides the scalar queue, which is guaranteed
                        # idle after the final exp (gpsimd may still be
                        # draining and would delay the tail out-projection).
                        eng = nc.scalar if (p == 1 and qg == NQG - 1) else nc.gpsimd
                        eng.dma_start(ctxT[p][DK:128, q0 : q0 + 512], cn)
                        if p == 1:
                            for st in range(qg * 4, qg * 4 + 4):
                                pe_aux.append((4, 0, outproj_closure(st)))
                return run

            def pv_slot(prev, kt: int):
                _, pp, pexps, pcps = prev
                for hh in range(2):
                    nc.tensor.matmul(
                        pcps[hh],
                        lhsT=vh2[:, kt, pp * 2 + hh, :],
                        rhs=pexps[kt][:, hh * 512 : (hh + 1) * 512],
                        start=(kt == 0),
                        stop=(kt == NST - 1),
                    )

            def finish_norm(prev):
                pqg, pp, _, pcps = prev
                for hh in range(2):
                    cps_sb = csbp.tile([DK + 1, 512], F32, tag=f"csb{hh}", name="cps_sb")
                    nc.vector.tensor_copy(cps_sb, pcps[hh])
                    stat = stp.tile([DK + 1, 512], F32, tag="stat", name="stat")
                    # NB: reciprocal_approx_fast mis-addresses single-row APs at
                    # base partition 64; run it over the full tile (same cost --
                    # partitions are parallel) and use only row 64.
                    nc.vector.reciprocal_approx_fast(stat, cps_sb)
                    statb = stp.tile([DK + 1, 512], BF16, tag="statb", name="statb")
                    nc.vector.tensor_copy(statb[DK : DK + 1, :], stat[DK : DK + 1, :])
                    pe_aux.append((1, 0, norm_closure(pqg, pp, hh, cps_sb, statb)))

            # ---- prologue -----------------------------------------------
            # Warm the PE HAM (4096-cycle activity window) with throwaway
            # matmuls while the first x chunks stream in, so the real
            # projections run at 2.4GHz instead of 1.2.
            warm = auxp.tile([128, 512], F32, tag="aux", name="warm")
            for _ in range(56):
                nc.tensor.matmul(
                    warm[0:DK, 0:DK], lhsT=ones64, rhs=ones64, start=True, stop=True
                )
            # pair-0 chunk-0 projections gate the first scores
            qk_chunk(khT[0], wk_s, 0, xk_t, 0, bias=False)()
            qk_chunk(qhT[0], wq_s, 0, xq_t, 0, bias=True)()

            # ---- deferred projection schedule (FIFO, deadline-ordered) --
            # (cost, min_unit, closure): min_unit defers K/Q pair-1 chunks
            # into the p=1 phase, where the exp stream paces the PE and
            # leaves it idle slack; p=0 units stay lean.
            sched = [
                (8, 0, qk_chunk(khT[0], wk_s, 0, xk_t, 1, False)),
                (8, 0, qk_chunk(khT[0], wk_s, 0, xk_t, 2, False)),
                (8, 0, qk_chunk(khT[0], wk_s, 0, xk_t, 3, False)),
                (9, 0, qk_chunk(qhT[0], wq_s, 0, xq_t, 1, True)),
                (8, 0, v_chunk(0)),
                (8, 0, v_chunk(1)),
                (9, 0, qk_chunk(qhT[0], wq_s, 0, xq_t, 2, True)),
                (8, 1, v_chunk(2)),
                (8, 1, v_chunk(3)),
                (8, 1, v_chunk(4)),
                (8, 1, v_chunk(5)),
                (8, 1, v_chunk(6)),
                (8, 1, v_chunk(7)),
                (9, 1, qk_chunk(qhT[0], wq_s, 0, xq_t, 3, True)),
                (8, 2, qk_chunk(khT[1], wk_s, 1, xk_t, 0, False)),
                (8, 2, qk_chunk(khT[1], wk_s, 1, xk_t, 1, False)),
                (9, 3, qk_chunk(qhT[1], wq_s, 1, xq_t, 0, True)),
                (8, 4, qk_chunk(khT[1], wk_s, 1, xk_t, 2, False)),
                (8, 4, qk_chunk(khT[1], wk_s, 1, xk_t, 3, False)),
                (9, 4, qk_chunk(qhT[1], wq_s, 1, xq_t, 1, True)),
                (9, 5, qk_chunk(qhT[1], wq_s, 1, xq_t, 2, True)),
                (9, 6, qk_chunk(qhT[1], wq_s, 1, xq_t, 3, True)),
            ]
            pe_aux.extend(sched)

            # ---- attention units, software-pipelined --------------------
            units = [(qg, p) for p in range(2) for qg in range(NQG)]
            # In late (p=1) units, drain the previous unit's PV at 1.5
            # kts/slot so its norm + out-projection run in-unit instead of
            # piling into the kernel tail.
            fast_pv = {
                0: [0, 1], 1: [2], 2: [3, 4], 3: [5], 4: [6, 7], 5: [8],
                6: [9, 10], 7: [11], 8: [12, 13], 9: [14], 10: [15],
            }
            prev = None
            for ui, (qg, p) in enumerate(units):
                cur_ui[0] = ui
                q0 = qg * 512
                grant = 4 if (ui <= 1 or ui >= 6) else 3
                fast = ui >= 5
                prev_done = False
                exps = []
                for kt in range(NST):
                    sc = scp.tile([128, 1024], F32, tag="sc", name="sc")
                    nc.tensor.matmul(
                        sc[:, 0:512],
                        lhsT=khT[p][0:64, kt * 128 : (kt + 1) * 128],
                        rhs=qhT[p][0:64, q0 : q0 + 512],
                        start=True,
                        stop=True,
                        tile_position=(0, 0),
                    )
                    nc.tensor.matmul(
                        sc[:, 512:1024],
                        lhsT=khT[p][64:128, kt * 128 : (kt + 1) * 128],
                        rhs=qhT[p][64:128, q0 : q0 + 512],
                        start=True,
                        stop=True,
                        tile_position=(64, 0),
                    )
                    et = epool.tile([128, 1024], BF16, tag="expT", name="expT")
                    nc.scalar.activation(et, sc, AF.Exp)
                    exps.append(et)
                    if prev is not None:
                        pkts = fast_pv.get(kt, []) if fast else [kt]
                        for pkt in pkts:
                            pv_slot(prev, pkt)
                        if fast and kt == 10:
                            finish_norm(prev)
                            prev_done = True
                    drain_aux(grant)
                if prev is not None and not prev_done:
                    finish_norm(prev)
                if ui == len(units) - 1:
                    # Last unit: take ctx PSUM from the (now idle) score
                    # pool so the tail's PV never waits on the previous
                    # unit's ctx-bank release chain.
                    cps = [
                        scp.tile([DK + 1, 512], F32, tag="sc", name="cps_tail")
                        for hh in range(2)
                    ]
                else:
                    cps = [
                        ctxp.tile([DK + 1, 512], F32, tag=f"c{hh}", name="cps")
                        for hh in range(2)
                    ]
                prev = (qg, p, exps, cps)

            # Tail: PV + norm + remaining out projections of the last unit.
            cur_ui[0] = len(units)
            for kt in range(NST):
                pv_slot(prev, kt)
                drain_aux(3)
            finish_norm(prev)
            # The last norm chain is ~2us of serial DVE work with no PE
            # load; keep the HAM activity window busy so the final out
            # projections run at 2.4GHz instead of re-throttled 1.2.
            # (Fresh tile: the prologue's warm tile slot was recycled.)
            warm2 = auxp.tile([128, 512], F32, tag="aux", name="warm2")
            for _ in range(36):
                nc.tensor.matmul(
                    warm2[0:DK, 0:DK], lhsT=ones64, rhs=ones64, start=True, stop=True
                )
            while pe_aux:
                pe_aux.popleft()[2]()

    nc.compile()
    return nc


_CACHE: dict = {}


def _get_nc() -> bass.Bass:
    if "nc" not in _CACHE:
        _CACHE["nc"] = build_nc()
    return _CACHE["nc"]


def _bf16(x: np.ndarray) -> np.ndarray:
    return np.ascontiguousarray(x).astype(ml_dtypes.bfloat16)


def _chunked(xT: np.ndarray, nch: int, cw: int) -> np.ndarray:
    # x^T [D, S] -> [128, nch, KT_D, cw] with [p, c, t, n] = x^T[t*128+p, c*cw+n]
    return np.ascontiguousarray(
        xT.reshape(KT_D, 128, nch, cw).transpose(1, 2, 0, 3)
    )


def make_in_maps(q, k, v, wq, bq, wk, bk, wv, bv, wo, bo):
    scale = np.float32(1.0 / np.sqrt(DK))
    in_maps = []
    xT = {}
    for b in range(B):
        xT[b] = (
            _chunked(_bf16(np.asarray(q[b], np.float32).T), NCH, CW),
            _chunked(_bf16(np.asarray(k[b], np.float32).T), NCH, CW),
            _chunked(_bf16(np.asarray(v[b], np.float32).T), VCH, VCW),
        )
    for c in range(NCORES):
        b, g = divmod(c, 4)
        hh = g * HD
        in_maps.append(
            {
                "xqC": xT[b][0],
                "xkC": xT[b][1],
                "xvC": xT[b][2],
                "wqT": _bf16(np.asarray(wq[hh : hh + HD], np.float32).T * scale),
                "wkT": _bf16(np.asarray(wk[hh : hh + HD], np.float32).T),
                "wvT": _bf16(np.asarray(wv[hh : hh + HD], np.float32).T),
                "woT": _bf16(np.asarray(wo[:, hh : hh + HD], np.float32).T),
                "bq": _bf16(np.asarray(bq[hh : hh + HD], np.float32) * scale),
            }
        )
    return in_maps


def assemble(results, bv, bo, wo) -> np.ndarray:
    out = np.zeros((B, S, D), np.float32)
    for c in range(NCORES):
        out[c // 4] += np.asarray(results[c]["out_partial"], np.float32)
    corr = np.asarray(bv, np.float32) @ np.asarray(wo, np.float32).T + np.asarray(
        bo, np.float32
    )
    out += corr[None, None, :]
    return out


def kernel(q, k, v, wq, bq, wk, bk, wv, bv, wo, bo) -> np.ndarray:
    nc = _get_nc()
    in_maps = make_in_maps(q, k, v, wq, bq, wk, bk, wv, bv, wo, bo)
    res = run_bass_kernel_spmd(nc, in_maps, list(range(NCORES))).results
    return assemble(res, bv, bo, wo)
